# revision 1
# baseline (speedup 1.0000x reference)
"""Trainium2 Bass kernel for nn_CLUBCategorical (CLUB categorical loss).

Reference computation:
    h      = relu(x @ W1 + b1)              [N, H]
    logits = h @ W2 + b2                    [N, Y]
    logp   = log_softmax(logits, -1)        [N, Y]
    out[i] = logp[i, y_i] - mean_j logp[i, y_j]

Algebraic simplification: with c[y] = histogram(y_idx), the log-softmax
normalizer cancels between the positive and negative terms:

    out[i] = L[i, y_i] - (1/N) * (L[i, :] @ c) + (b2[y_i] - (b2 @ c)/N)

where L = relu(x @ W1 + b1) @ W2 (no bias, no softmax). On device this is
two dense matmuls plus a masked column reduction:

    out[i] = sum_y L[i, y] * (onehot(y_i)[y] - c[y]/N) + g[i]

Sharding: data-parallel over N. Each of the 8 cores handles 1024 rows and
gets the full W1/W2 plus the global label histogram (the "all-gather of
column labels" is folded into c on the host). No collectives needed.

Device layout (per core; contraction dim always on SBUF partitions, all
operand layouts pre-arranged on host so every DMA is one contiguous
descriptor):
    phase 1: hT[m]  [128h, 1024r] = W1[:,mslice].T @ xT[:, rows] (+b1, relu)
    phase 2: psum_l [128y,  512r] = W2[:,qslice].T @ hT[:, rows]
             eqc    [128y,  512r] = (ybc == iota_q) - cN_q       (DVE)
             prod   = psum_l * eqc                               (DVE)
             out    += ones.T @ prod  (M=1 matmul reduces over y) (PE)
ybc is broadcast on device from a [1, rows] vector via a K=1 matmul.
Matmuls run in float32r (~2^-13 relative precision, 2x fp32 throughput).
DMA descriptors are interleaved across the two HWDGE queues (sync,
scalar) in phase-1 consumption order; constants ride the gpsimd SWDGE.
"""

import numpy as np

N, X_DIM, Y_DIM, HIDDEN = 8192, 512, 512, 1024
N_CORES = 8
N_LOC = N // N_CORES          # 1024 rows per core
KX = X_DIM // 128             # 4  k-chunks, phase 1
KH = HIDDEN // 128            # 8  k-chunks, phase 2 / m-chunks, phase 1
QY = Y_DIM // 128             # 4  y-chunks, phase 2
RG = N_LOC // 512             # 2  row groups of 512

_NC_CACHE = {}


def _build(nc_cls, mybir, tile):
    mdt = mybir.dt
    f32 = mdt.float32
    F32R = mdt.float32r
    AF = mybir.ActivationFunctionType
    OP = mybir.AluOpType

    nc = nc_cls("TRN2", target_bir_lowering=False, debug=False,
                num_devices=N_CORES)

    # xt{n}{a,b}: x rows for row-group n, partition-major, k-halves
    xtD = [[nc.dram_tensor(f"xt{n}{h}", [128, 2 * 512], f32,
                           kind="ExternalInput") for h in "ab"]
           for n in range(RG)]
    # w1p{mp}: W1 columns for hidden-pair mp, all K
    w1D = [nc.dram_tensor(f"w1p{mp}", [128, KX * 256], f32,
                          kind="ExternalInput") for mp in range(KH // 2)]
    # w2p{h}: W2 rows half h, partition-major
    w2D = [nc.dram_tensor(f"w2p{h}", [128, 4 * Y_DIM], f32,
                          kind="ExternalInput") for h in range(2)]
    # packed constants: [b1c(8) | iot(4) | cNc(4) | ones(1)] = [128, 17]
    cst = nc.dram_tensor("cst", [128, KH + 2 * QY + 1], f32,
                         kind="ExternalInput")
    o128 = nc.dram_tensor("o128", [1, 128], f32, kind="ExternalInput")
    yrow = nc.dram_tensor("yrow", [1, N_LOC], f32, kind="ExternalInput")
    gv = nc.dram_tensor("gv", [1, N_LOC], f32, kind="ExternalInput")
    out = nc.dram_tensor("out", [1, N_LOC], f32, kind="ExternalOutput")

    with tile.TileContext(nc) as tc:
        with (
            tc.tile_pool(name="wgt", bufs=1) as wgt,
            tc.tile_pool(name="hp", bufs=1) as hp,
            tc.tile_pool(name="eqp", bufs=1) as eqp,
            tc.tile_pool(name="prp", bufs=4) as prp,
            tc.tile_pool(name="osb", bufs=1) as osb,
            tc.tile_pool(name="ps", bufs=1, space="PSUM") as ps,
        ):
            cst_sb = wgt.tile([128, KH + 2 * QY + 1], F32R, tag="cst")
            b1_sb = cst_sb[:, 0:KH].bitcast(f32)
            iot_sb = cst_sb[:, KH:KH + QY].bitcast(f32)
            cnc_sb = cst_sb[:, KH + QY:KH + 2 * QY].bitcast(f32)
            ones_sb = cst_sb[:, KH + 2 * QY:KH + 2 * QY + 1]
            yrow_sb = wgt.tile([1, N_LOC], F32R, tag="yrow")
            o128_sb = wgt.tile([1, 128], F32R, tag="o128")
            g_sb = wgt.tile([1, N_LOC], f32, tag="g")
            nc.gpsimd.dma_start(g_sb[:], gv.ap())

            # --- big loads, interleaved across both HWDGE queues in
            # phase-1 consumption order ---
            xt_sb = [wgt.tile([128, KX * 512], F32R, tag=f"xt_{n}",
                              name=f"xt_{n}") for n in range(RG)]
            w1p_sb = [wgt.tile([128, KX * 256], F32R, tag=f"w1_{mp}",
                               name=f"w1_{mp}") for mp in range(KH // 2)]
            w2p_sb = [wgt.tile([128, 4 * Y_DIM], F32R, tag=f"w2p_{h}",
                               name=f"w2p_{h}") for h in range(2)]
            # Arrival schedule (each queue ~166GB/s, ~3us per 512KB):
            # sync:   xt0a@12 w1p1@15 w1p3@18 w2p0a@21 xt1a@24 w2p1a@27
            # scalar: xt0b@12 w1p0@12+ w1p2@18 w2p0b@21 xt1b@24 w2p1b@27
            # matching PE consumption: p1(n0) -> p2(n0) j0..3 -> p1(n1)
            # -> p2(n0) j4..7 -> p2(n1); only 16 MMs depend on the last MB.
            nc.sync.dma_start(xt_sb[0][:, 0:1024],
                              xtD[0][0].ap().bitcast(F32R))
            nc.scalar.dma_start(w1p_sb[0][:], w1D[0].ap().bitcast(F32R))
            nc.scalar.dma_start(xt_sb[0][:, 1024:2048],
                                xtD[0][1].ap().bitcast(F32R))
            nc.sync.dma_start(yrow_sb[:], yrow.ap().bitcast(F32R))
            nc.sync.dma_start(o128_sb[:], o128.ap().bitcast(F32R))
            nc.scalar.dma_start(cst_sb[:], cst.ap().bitcast(F32R))
            nc.sync.dma_start(w1p_sb[1][:], w1D[1].ap().bitcast(F32R))
            nc.scalar.dma_start(w1p_sb[2][:], w1D[2].ap().bitcast(F32R))
            nc.sync.dma_start(w1p_sb[3][:], w1D[3].ap().bitcast(F32R))
            nc.sync.dma_start(w2p_sb[0][:, 0:1024],
                              w2D[0].ap()[:, 0:1024].bitcast(F32R))
            nc.scalar.dma_start(w2p_sb[0][:, 1024:2048],
                                w2D[0].ap()[:, 1024:2048].bitcast(F32R))
            nc.sync.dma_start(xt_sb[1][:, 0:1024],
                              xtD[1][0].ap().bitcast(F32R))
            nc.scalar.dma_start(xt_sb[1][:, 1024:2048],
                                xtD[1][1].ap().bitcast(F32R))
            nc.sync.dma_start(w2p_sb[1][:, 0:1024],
                              w2D[1].ap()[:, 0:1024].bitcast(F32R))
            nc.scalar.dma_start(w2p_sb[1][:, 1024:2048],
                                w2D[1].ap()[:, 1024:2048].bitcast(F32R))
            w2_sb = [w2p_sb[j // 4][:, (j % 4) * Y_DIM:(j % 4 + 1) * Y_DIM]
                     for j in range(KH)]

            def w1_slice(k, m):
                mp, mo = m // 2, m % 2
                return w1p_sb[mp][:, k * 256 + mo * 128:
                                  k * 256 + (mo + 1) * 128]

            def xt_slice(k, n):
                return xt_sb[n][:, k * 512:(k + 1) * 512]

            # pout accumulators reuse the yb slots (free after eqc)
            pout = {n: ps.tile([1, 512], f32, tag="yb", bufs=RG,
                               name=f"po_{n}") for n in range(RG)}

            hT = [hp.tile([128, N_LOC], F32R, tag=f"h_{j}", name=f"h_{j}")
                  for j in range(KH)]

            def phase1(n):
                for m in range(KH):
                    psum = ps.tile([128, 512], f32, tag="psum", bufs=6,
                                   name=f"p1_{n}_{m}")
                    for k in range(KX):
                        nc.tensor.matmul(
                            psum[:], w1_slice(k, m), xt_slice(k, n),
                            start=(k == 0), stop=(k == KX - 1))
                    nc.scalar.activation(
                        hT[m][:, n * 512:(n + 1) * 512], psum[:],
                        AF.Relu, bias=b1_sb[:, m:m + 1])

            def p2_mms(psum_l, n, q, j0, j1):
                for j in range(j0, j1):
                    nc.tensor.matmul(
                        psum_l[:],
                        w2_sb[j][:, q * 128:(q + 1) * 128],
                        hT[j][:, n * 512:(n + 1) * 512],
                        start=(j == 0), stop=(j == KH - 1))

            pending = []  # delay ones-MMs so PE never waits on DVE prod

            def finish_group(psum_l, n, q):
                prod = prp.tile([128, 512], F32R, name=f"prod_{n}_{q}")
                nc.vector.tensor_tensor(
                    prod[:], psum_l[:], eqc_sb[(n, q)][:], OP.mult)
                pending.append((n, q, prod))

            def flush_one():
                n, q, prod = pending.pop(0)
                nc.tensor.matmul(
                    pout[n][:], ones_sb, prod[:],
                    start=(q == 0), stop=(q == QY - 1))

            # pipelined schedule matched to DMA arrivals.
            # First: 8 K=128 warmup matmuls on a self-produced tile with
            # ZERO DMA dependencies (memset -> DVE round to f32r), so they
            # run during the otherwise-dead preamble window and the PE is
            # already at the warm 2.4GHz clock when the first real
            # operands land.
            # warmup source produced on-chip (memset -> DVE round to
            # f32r): no DMA dependency, so the warmup matmuls run during
            # the otherwise-dead preamble/DMA-lead-in window
            wu_f32 = wgt.tile([128, 512], f32, tag="wuf")
            nc.gpsimd.memset(wu_f32[:], 1.0)
            wu_src = wgt.tile([128, 512], F32R, tag="wur")
            nc.vector.tensor_copy(wu_src[:], wu_f32[:])
            wu = ps.tile([128, 512], f32, tag="psum", bufs=6, name="wu")
            for _ in range(8):
                nc.tensor.matmul(wu[:], wu_src[:, 0:128],
                                 wu_src[:], start=True, stop=True)
            phase1(0)
            # ybc broadcast: K=1 matmul replicates yrow across partitions;
            # eqc masks read it straight from PSUM (needed at finish_group)
            eqc_sb = {}
            for n in range(RG):
                yb = ps.tile([128, 512], f32, tag="yb", bufs=RG,
                             name=f"yb{n}")
                nc.tensor.matmul(
                    yb[:], o128_sb[:],
                    yrow_sb[:, n * 512:(n + 1) * 512],
                    start=True, stop=True)
                for q in range(QY):
                    e = eqp.tile([128, 512], f32, tag=f"eqc_{n}_{q}",
                                 name=f"eqc_{n}_{q}")
                    nc.vector.tensor_scalar(
                        e[:], yb[:], iot_sb[:, q:q + 1], cnc_sb[:, q:q + 1],
                        OP.is_equal, OP.subtract)
                    eqc_sb[(n, q)] = e
            pl_n0 = [ps.tile([128, 512], f32, tag="psum", bufs=6,
                             name=f"pl_0_{q}") for q in range(QY)]
            for q in range(QY):           # needs only w2p0 (j0..3)
                p2_mms(pl_n0[q], 0, q, 0, KH // 2)
            phase1(1)                     # needs xt1; w2p1 streams behind
            for q in range(QY):           # finish n0 with w2p1 (j4..7)
                p2_mms(pl_n0[q], 0, q, KH // 2, KH)
                finish_group(pl_n0[q], 0, q)
            for q in range(QY):
                psum_l = ps.tile([128, 512], f32, tag="psum", bufs=6,
                                 name=f"pl_1_{q}")
                p2_mms(psum_l, 1, q, 0, KH)
                finish_group(psum_l, 1, q)
                flush_one()
            while pending:
                flush_one()

            # --- epilogue: add g, store (single output DMA) ---
            o = osb.tile([1, N_LOC], f32, tag="o")
            for n in range(RG):
                nc.vector.tensor_tensor(
                    o[:, n * 512:(n + 1) * 512], pout[n][:],
                    g_sb[:, n * 512:(n + 1) * 512], OP.add)
            nc.sync.dma_start(out.ap(), o[:])

    nc.compile()
    return nc


def _get_nc():
    if "nc" not in _NC_CACHE:
        import concourse.bacc as bacc
        import concourse.mybir as mybir
        from concourse import tile
        _NC_CACHE["nc"] = _build(bacc.Bacc, mybir, tile)
    return _NC_CACHE["nc"]


def kernel(x_samples, y_idx, W1, b1, W2, b2):
    from concourse.bass_utils import run_bass_kernel_spmd

    x = np.ascontiguousarray(np.asarray(x_samples, dtype=np.float32))
    y = np.asarray(y_idx).astype(np.int64).reshape(-1)
    W1 = np.ascontiguousarray(np.asarray(W1, dtype=np.float32))
    b1 = np.asarray(b1, dtype=np.float32).reshape(-1)
    W2 = np.ascontiguousarray(np.asarray(W2, dtype=np.float32))
    b2 = np.asarray(b2, dtype=np.float32).reshape(-1)

    # global label histogram + fully-folded bias term
    c = np.bincount(y, minlength=Y_DIM).astype(np.float32)
    cN = c / np.float32(N)
    beta = np.float32(b2 @ c) / np.float32(N)
    g_full = (b2[y] - beta).astype(np.float32)

    # device layouts: every DMA is one contiguous descriptor
    # w1_dev[mp][p, k*256+c] = W1[k*128+p, mp*256+c]
    w1_dev = np.ascontiguousarray(
        W1.reshape(KX, 128, KH // 2, 256).transpose(2, 1, 0, 3)
        .reshape(KH // 2, 128, KX * 256))
    # w2_dev[h][p, a*512+y] = W2[(h*4+a)*128+p, y]
    w2_dev = np.ascontiguousarray(
        W2.reshape(2, 4, 128, Y_DIM).transpose(0, 2, 1, 3)
        .reshape(2, 128, 4 * Y_DIM))
    b1c = b1.reshape(KH, 128).T                                   # [128, 8]
    iot = np.arange(Y_DIM, dtype=np.float32).reshape(QY, 128).T   # [128, 4]
    cNc = cN.reshape(QY, 128).T                                   # [128, 4]
    onesv = np.ones((128, 1), dtype=np.float32)
    cst = np.ascontiguousarray(
        np.concatenate([b1c, iot, cNc, onesv], axis=1))           # [128, 17]
    o128 = np.ones((1, 128), dtype=np.float32)

    in_maps = []
    for m in range(N_CORES):
        sl = slice(m * N_LOC, (m + 1) * N_LOC)
        # xt_dev[n][p, k*512+r] = x[m*N_LOC + n*512+r, k*128+p]
        xt_dev = np.ascontiguousarray(
            x[sl].reshape(RG, 512, KX, 128).transpose(0, 3, 2, 1)
            .reshape(RG, 128, KX * 512))
        im = {
            **{f"w1p{mp}": w1_dev[mp] for mp in range(KH // 2)},
            **{f"w2p{h}": w2_dev[h] for h in range(2)},
            "cst": cst,
            "o128": o128,
            "yrow": np.ascontiguousarray(
                y[sl].astype(np.float32)).reshape(1, N_LOC),
            "gv": np.ascontiguousarray(g_full[sl]).reshape(1, N_LOC),
        }
        for n in range(RG):
            im[f"xt{n}a"] = np.ascontiguousarray(xt_dev[n][:, 0:1024])
            im[f"xt{n}b"] = np.ascontiguousarray(xt_dev[n][:, 1024:2048])
        in_maps.append(im)

    nc = _get_nc()
    res = run_bass_kernel_spmd(nc, in_maps, core_ids=list(range(N_CORES)))
    return np.concatenate(
        [res.results[m]["out"].reshape(-1) for m in range(N_CORES)]
    ).astype(np.float32)



# revision 7
# speedup vs baseline: 1.3269x; 1.3269x over previous
"""Trainium2 Bass kernel for nn_CLUBCategorical (CLUB categorical loss).

Reference computation:
    h      = relu(x @ W1 + b1)              [N, H]
    logits = h @ W2 + b2                    [N, Y]
    logp   = log_softmax(logits, -1)        [N, Y]
    out[i] = logp[i, y_i] - mean_j logp[i, y_j]

Algebraic reduction: the log-softmax normalizer cancels between the
positive and negative terms, and with c[y] = histogram(y_idx),
u = (W2 @ c)/N, beta = (b2 @ c)/N:

    out[i] = h[i,:] @ (W2[:, y_i] - u) + (b2[y_i] - beta)

so the dense phase-2 matmul h @ W2 collapses to a per-row dot product
against a host-gathered matrix GT[k, i] = W2[k, y_i] - u[k]. On device:

    phase 1: hT[m] [128h, 1024r] = relu(W1[:,m].T @ xT + b1)   (PE + ACT)
    phase 2: acc  += hT[m] (.) GT[m]   elementwise, DVE
             out   = ones.T @ acc      (2 ones-matmuls, PE)
    host:    out  += b2[y] - beta

Sharding: data-parallel over N; each core takes 1024 rows with full W1
and its own gathered GT. No collectives. All big operands travel in
bf16 (PE runs bf16 at the same 1 col/cycle as fp32r, so this halves
HBM traffic at no PE cost; tolerance is 2e-2, bf16 lands ~5e-3).
DMA split: x + second-half GT on the sync HWDGE queue, W1 + first-half
GT on the scalar queue, b1 on the gpsimd SWDGE. Warmup matmuls on a
memset tile cover the DMA lead-in and start the PE clock ramp early.
"""

import numpy as np

N, X_DIM, Y_DIM, HIDDEN = 8192, 512, 512, 1024
N_CORES = 8
N_LOC = N // N_CORES          # 1024 rows per core
KX = X_DIM // 128             # 4  k-chunks, phase 1
MH = HIDDEN // 128            # 8  hidden chunks
RG = N_LOC // 512             # 2  row groups of 512

_NC_CACHE = {}


def _build(nc_cls, mybir, tile):
    mdt = mybir.dt
    f32 = mdt.float32
    F32R = mdt.float32r
    BF16 = mdt.bfloat16
    AF = mybir.ActivationFunctionType
    OP = mybir.AluOpType

    nc = nc_cls("TRN2", target_bir_lowering=False, debug=False,
                num_devices=N_CORES)

    # xt[p, rg*2048 + k*512 + r] = x[rg*512 + r, k*128 + p]
    xtD = nc.dram_tensor("xt", [128, RG * KX * 512], BF16,
                         kind="ExternalInput")
    # w1t[p, m*512 + k*128 + c] = W1[k*128 + p, m*128 + c]
    w1D = nc.dram_tensor("w1t", [128, MH * 512], BF16, kind="ExternalInput")
    # gt[p, m*1024 + i] = W2[m*128 + p, y[i]] - u[m*128 + p]
    gtD = nc.dram_tensor("gt", [128, MH * N_LOC], BF16,
                         kind="ExternalInput")
    # b1c[p, m] = b1[m*128 + p]
    cstD = nc.dram_tensor("cst", [128, MH], f32, kind="ExternalInput")
    out = nc.dram_tensor("out", [1, N_LOC], f32, kind="ExternalOutput")

    with tile.TileContext(nc) as tc:
        with (
            tc.tile_pool(name="wgt", bufs=1) as wgt,
            tc.tile_pool(name="hp", bufs=4) as hp,
            tc.tile_pool(name="prp", bufs=4) as prp,
            tc.tile_pool(name="accp", bufs=1) as accp,
            tc.tile_pool(name="osb", bufs=1) as osb,
            tc.tile_pool(name="ps", bufs=1, space="PSUM") as ps,
        ):
            # on-chip constants: warmup source + ones column (no DMA deps;
            # the verifier requires fp32r matmul operands to be written as
            # F32R, hence the memset -> copy)
            wu_f32 = wgt.tile([128, 512], f32, tag="wuf")
            nc.vector.memset(wu_f32[:], 1.0)
            wu_src = wgt.tile([128, 512], F32R, tag="wur")
            nc.vector.tensor_copy(wu_src[:], wu_f32[:])
            wu_op = wu_src[:]
            ones_f = wgt.tile([128, 1], f32, tag="onesf")
            nc.vector.memset(ones_f[:], 1.0)
            ones_t = wgt.tile([128, 1], F32R, tag="ones")
            nc.vector.tensor_copy(ones_t[:], ones_f[:])

            cst_sb = wgt.tile([128, MH], f32, tag="cst")
            nc.gpsimd.dma_start(cst_sb[:], cstD.ap())

            xt_sb = wgt.tile([128, RG * KX * 512], BF16, tag="xt")
            w1_sb = wgt.tile([128, MH * 512], BF16, tag="w1")
            gt_sb = wgt.tile([128, MH * N_LOC], BF16, tag="gt")

            # sync queue: x (rg0 k0 first, small, to unblock the PE), then
            # the rest of x, then the second half of GT.
            nc.sync.dma_start(xt_sb[:, 0:512], xtD.ap()[:, 0:512])
            nc.scalar.dma_start(w1_sb[:, 0:1024], w1D.ap()[:, 0:1024])
            nc.sync.dma_start(xt_sb[:, 512:2048], xtD.ap()[:, 512:2048])
            nc.scalar.dma_start(w1_sb[:, 1024:2048], w1D.ap()[:, 1024:2048])
            nc.sync.dma_start(xt_sb[:, 2048:3072], xtD.ap()[:, 2048:3072])
            nc.scalar.dma_start(w1_sb[:, 2048:3072], w1D.ap()[:, 2048:3072])
            nc.sync.dma_start(xt_sb[:, 3072:4096], xtD.ap()[:, 3072:4096])
            nc.scalar.dma_start(w1_sb[:, 3072:4096], w1D.ap()[:, 3072:4096])
            nc.scalar.dma_start(gt_sb[:, 0:2048], gtD.ap()[:, 0:2048])
            nc.sync.dma_start(gt_sb[:, 4096:6144], gtD.ap()[:, 4096:6144])
            nc.scalar.dma_start(gt_sb[:, 2048:4096], gtD.ap()[:, 2048:4096])
            nc.sync.dma_start(gt_sb[:, 6144:8192], gtD.ap()[:, 6144:8192])

            def xt_slice(k, rg):
                o = rg * 2048 + k * 512
                return xt_sb[:, o:o + 512]

            def w1_slice(k, m):
                o = m * 512 + k * 128
                return w1_sb[:, o:o + 128]

            def gt_slice(m, rg):
                o = m * N_LOC + rg * 512
                return gt_sb[:, o:o + 512]

            acc = [accp.tile([128, 512], F32R, tag=f"acc{rg}",
                             name=f"acc{rg}") for rg in range(RG)]

            # warmup: keep the PE busy during the DMA lead-in so the
            # clock ramp starts as early as possible
            wu = ps.tile([128, 512], f32, tag="psum", bufs=6, name="wu")
            for _ in range(5):
                nc.tensor.matmul(wu[:], wu_op[:, 0:128], wu_op[:],
                                 start=True, stop=True)

            def chunk(m, rg, last):
                psum = ps.tile([128, 512], f32, tag="psum", bufs=6,
                               name=f"p_{rg}_{m}")
                for k in range(KX):
                    nc.tensor.matmul(psum[:], w1_slice(k, m),
                                     xt_slice(k, rg),
                                     start=(k == 0), stop=(k == KX - 1))
                if last:
                    # final chunk: relu+bias on DVE (shorter latency than
                    # the ACT path; this chain is the kernel tail)
                    h = hp.tile([128, 512], BF16, name=f"h_{rg}_{m}")
                    nc.vector.tensor_scalar(
                        h[:], psum[:], cst_sb[:, m:m + 1], 0.0,
                        OP.add, OP.max)
                else:
                    h = hp.tile([128, 512], BF16, name=f"h_{rg}_{m}")
                    nc.scalar.activation(h[:], psum[:], AF.Relu,
                                         bias=cst_sb[:, m:m + 1])
                if m == 0:
                    nc.vector.tensor_tensor(
                        acc[rg][:], h[:], gt_slice(m, rg), OP.mult)
                else:
                    prod = prp.tile([128, 512], BF16, name=f"pr_{rg}_{m}")
                    nc.vector.tensor_tensor(
                        prod[:], h[:], gt_slice(m, rg), OP.mult)
                    nc.vector.tensor_tensor(
                        acc[rg][:], acc[rg][:], prod[:], OP.add)

            o = osb.tile([1, N_LOC], f32, tag="o")
            pout = [ps.tile([1, 512], f32, tag=f"po{rg}", bufs=1,
                            name=f"po_{rg}") for rg in range(RG)]

            for m in range(MH):
                chunk(m, 0, last=False)
            chunk(0, 1, last=False)
            # rg0's reduction: emitted after one rg1 chunk so the PE
            # never waits on the DVE accumulation
            nc.tensor.matmul(pout[0][:], ones_t[:], acc[0][:],
                             start=True, stop=True)
            nc.scalar.activation(o[:, 0:512], pout[0][:], AF.Copy)
            nc.scalar.dma_start(out.ap()[:, 0:512], o[:, 0:512])
            for m in range(1, MH):
                chunk(m, 1, last=(m == MH - 1))
            nc.tensor.matmul(pout[1][:], ones_t[:], acc[1][:],
                             start=True, stop=True)
            nc.vector.tensor_copy(o[:, 512:1024], pout[1][:])
            nc.sync.dma_start(out.ap()[:, 512:1024], o[:, 512:1024])

    nc.compile()
    return nc


def _get_nc():
    if "nc" not in _NC_CACHE:
        import concourse.bacc as bacc
        import concourse.mybir as mybir
        from concourse import tile
        _NC_CACHE["nc"] = _build(bacc.Bacc, mybir, tile)
    return _NC_CACHE["nc"]


def kernel(x_samples, y_idx, W1, b1, W2, b2):
    import ml_dtypes
    from concourse.bass_utils import run_bass_kernel_spmd

    bf16 = ml_dtypes.bfloat16
    x = np.ascontiguousarray(np.asarray(x_samples, dtype=np.float32))
    y = np.asarray(y_idx).astype(np.int64).reshape(-1)
    W1 = np.ascontiguousarray(np.asarray(W1, dtype=np.float32))
    b1 = np.asarray(b1, dtype=np.float32).reshape(-1)
    W2 = np.ascontiguousarray(np.asarray(W2, dtype=np.float32))
    b2 = np.asarray(b2, dtype=np.float32).reshape(-1)

    # host-side algebra: label histogram folds the negative term into u,
    # the bias terms fold into g (added back on host)
    c = np.bincount(y, minlength=Y_DIM).astype(np.float32)
    u = (W2 @ c) / np.float32(N)                                  # [H]
    beta = np.float32(b2 @ c) / np.float32(N)
    g_full = (b2[y] - beta).astype(np.float32)                    # [N]

    # w1t[p, m*512 + k*128 + c] = W1[k*128+p, m*128+c]
    w1_dev = np.ascontiguousarray(
        W1.reshape(KX, 128, MH, 128).transpose(1, 2, 0, 3)
        .reshape(128, MH * 512)).astype(bf16)
    b1c = np.ascontiguousarray(b1.reshape(MH, 128).T)             # [128, 8]
    # gathered, recentered W2 columns: GT[k, i] = W2[k, y_i] - u[k]
    gt_all = (W2[:, y] - u[:, None]).astype(bf16)                 # [H, N]

    in_maps = []
    for mcore in range(N_CORES):
        sl = slice(mcore * N_LOC, (mcore + 1) * N_LOC)
        xt_dev = np.ascontiguousarray(
            x[sl].reshape(RG, 512, KX, 128).transpose(3, 0, 2, 1)
            .reshape(128, RG * KX * 512)).astype(bf16)
        gt_dev = np.ascontiguousarray(
            gt_all[:, sl].reshape(MH, 128, N_LOC).transpose(1, 0, 2)
            .reshape(128, MH * N_LOC))
        in_maps.append({"xt": xt_dev, "w1t": w1_dev, "gt": gt_dev,
                        "cst": b1c})

    nc = _get_nc()
    res = run_bass_kernel_spmd(nc, in_maps, core_ids=list(range(N_CORES)))
    return np.concatenate(
        [res.results[m]["out"].reshape(-1) + g_full[m * N_LOC:(m + 1) * N_LOC]
         for m in range(N_CORES)]).astype(np.float32)
